# revision 6
# baseline (speedup 1.0000x reference)
"""Multi-head attention kernel for Trainium2, data-parallel over batch on 8 cores.

Problem: B=16, N=1024, DIM=768, H=12 heads, head_dim=64, fp32.
  q = x@Wq+bq; k = x@Wk+bk; v = x@Wv+bv   (per-head split)
  out = softmax(q k^T / sqrt(DIM)) v      (per head), concat, @Wo + bo

Sharding: batch-parallel. Each core gets 2 batches and all weights; no
collectives. Output gathered by concat.

Per-core layout strategy (per batch of 1024 tokens):
  - XT = x^T  [768 feat, 1024 tok] via PE transposes (fp32 DMA transpose
    unsupported).
  - QT/KT = (x@W + b)^T [768, 1024]: matmul(lhsT=W, rhs=XT). Head h lives on
    partition rows (h%2)*64..: pair p = m-tile p.
  - V natural [1024 tok, 768] via matmul(lhsT=XT, rhs=Wv), stored per-pair
    padded: [Vh0(64) | ones(1) | Vh1(64)] = 129 cols (+pad). The shared ones
    column makes PV emit softmax denominators:
      h0: lhsT cols [0:65]  -> psum rows 0-63 = O_h0^T, row 64 = denom_h0
      h1: lhsT cols [1:129] -> psum row 63 = denom_h1, rows 64-127 = O_h1^T
         (rows 0-62 garbage, never read)
  - S^T[key, q] = matmul(lhsT=KT head rows, rhs=QT head rows), contraction 64,
    two heads row-packed in the PE array (partitions 0-63 / 64-127).
  - P^T = exp(SCALE * S^T) on ACT (no max subtraction needed: |SCALE*S| < ~2),
    [128, 1024] ops (2 key-blocks per op) to amortize ACT overhead.
  - O^T normalized by broadcast reciprocal rows, written to OT [768, 1024].
  - Y = matmul(lhsT=OT, rhs=Wo) + bo -> natural [tok, 768], DMA out.

Matmuls run in float32r (fp32 bits, reduced-precision PE mode, 4x faster than
fp32 mode; measured elementwise error ~1e-3 max).
"""

import sys
import types

sys.path.insert(0, "/opt/trn_rl_repo")

import numpy as np

# Register the axon NTFF profile hook if the image's antenv lacks it (needed
# only when run with trace=True; harmless otherwise).
import antenv  # noqa: F401

if "antenv.axon_hooks" not in sys.modules:
    _hooks_mod = types.ModuleType("antenv.axon_hooks")
    _hooks_mod._hook = None

    def _set_hook(h):
        _hooks_mod._hook = h

    def _get_hook():
        return _hooks_mod._hook

    _hooks_mod.set_axon_ntff_profile_hook = _set_hook
    _hooks_mod.get_axon_ntff_profile_hook = _get_hook
    sys.modules["antenv.axon_hooks"] = _hooks_mod
    try:
        from trn_agent_boot.trn_boot import _ntff_profile_via_ctypes

        _set_hook(_ntff_profile_via_ctypes("/opt/axon/libaxon_pjrt.so"))
    except Exception:
        pass

import concourse.bass_utils as bass_utils

bass_utils.upload_artifacts = lambda tmpdir: f"local:{tmpdir}"  # no bucket creds

import concourse.bacc as bacc
import concourse.mybir as mybir
import concourse.tile as tile
from concourse.bass_utils import run_bass_kernel_spmd
from concourse.masks import make_identity

P = 128
DIM = 768
N_HEADS = 12
HD = 64
N = 1024
B = 16
NCORES = 8
BL = B // NCORES  # batches per core = 2
SCALE = 1.0 / float(np.sqrt(DIM))

KT = DIM // P      # 6 k-tiles of the 768 contraction
TT = N // P        # 8 token tiles per batch
NPAIR = N_HEADS // 2  # 6 head pairs
QC = 512           # query chunk (psum bank, fp32)
PAIRW = 160        # pair block in V_ext: [Vh0(64)|ones(1)|pad(31)|Vh1(64)]

F32 = mybir.dt.float32

_cache = {}


def build(mm_dtype):
    nc = bacc.Bacc("TRN2", target_bir_lowering=False, debug=False)

    x = nc.dram_tensor("inputs", [BL, N, DIM], F32, kind="ExternalInput")
    wq = nc.dram_tensor("Wq", [DIM, DIM], F32, kind="ExternalInput")
    bq = nc.dram_tensor("bq", [DIM], F32, kind="ExternalInput")
    wk = nc.dram_tensor("Wk", [DIM, DIM], F32, kind="ExternalInput")
    bk = nc.dram_tensor("bk", [DIM], F32, kind="ExternalInput")
    wv = nc.dram_tensor("Wv", [DIM, DIM], F32, kind="ExternalInput")
    bv = nc.dram_tensor("bv", [DIM], F32, kind="ExternalInput")
    wo = nc.dram_tensor("Wo", [DIM, DIM], F32, kind="ExternalInput")
    bo = nc.dram_tensor("bo", [DIM], F32, kind="ExternalInput")
    out = nc.dram_tensor("out", [BL, N, DIM], F32, kind="ExternalOutput")

    wq_r = wq.rearrange("(ko ki) m -> ki ko m", ki=P)
    wk_r = wk.rearrange("(ko ki) m -> ki ko m", ki=P)
    wv_r = wv.rearrange("(ko ki) m -> ki ko m", ki=P)
    wo_r = wo.rearrange("(ko ki) m -> ki ko m", ki=P)
    bq_r = bq.rearrange("(ko ki) -> ki ko", ki=P)
    bk_r = bk.rearrange("(ko ki) -> ki ko", ki=P)

    # weights DMA: gpsimd can cast f32 -> f32r in flight
    wdma = nc.sync.dma_start if mm_dtype == F32 else nc.gpsimd.dma_start

    with tile.TileContext(nc) as tc:
        with (
            tc.tile_pool(name="const", bufs=1) as cpool,
            tc.tile_pool(name="work", bufs=1) as pool,
            tc.tile_pool(name="ps", bufs=1, space="PSUM") as ps,
        ):
            ident = cpool.tile([P, P], F32)
            make_identity(nc, ident)

            # resident weights: Wv, Wo (full); Wq/Wk streamed per pair below
            wv_sb = cpool.tile([P, KT, DIM], mm_dtype)
            wo_sb = cpool.tile([P, KT, DIM], mm_dtype)
            for k in range(KT):
                wdma(wv_sb[:, k], wv_r[:, k])
                wdma(wo_sb[:, k], wo_r[:, k])

            bq_sb = cpool.tile([P, KT], F32)
            bk_sb = cpool.tile([P, KT], F32)
            nc.sync.dma_start(bq_sb[:], bq_r)
            nc.sync.dma_start(bk_sb[:], bk_r)
            bv_b = cpool.tile([P, DIM], F32)
            bo_b = cpool.tile([P, DIM], F32)
            nc.sync.dma_start(bv_b[:], bv[None, :].to_broadcast((P, DIM)))
            nc.sync.dma_start(bo_b[:], bo[None, :].to_broadcast((P, DIM)))

            # V_ext: [tok_inner, tok_outer, pair blocks of PAIRW cols]
            # cols p*PAIRW + [0:64] = V head 2p, +64 = ones, +[96:160] = V 2p+1
            # pad cols stay uninitialized: they only produce garbage psum rows
            # that are never read. Ones col via DVE cast-copy (f32r producer).
            v_ext = cpool.tile([P, TT, NPAIR * PAIRW], mm_dtype)
            ones_src = cpool.tile([P, TT * NPAIR], F32)
            nc.vector.memset(ones_src[:], 1.0)
            ones_cols = v_ext[:].rearrange("p t (np w) -> p t np w", w=PAIRW)[
                :, :, :, 64:65
            ]
            nc.vector.tensor_copy(
                ones_cols,
                ones_src[:].rearrange("p (t np) -> p t np", np=NPAIR)[:, :, :, None],
            )

            for b in range(BL):
                # ---- XT = x[b]^T ------------------------------------------
                xt = pool.tile([P, KT, N], mm_dtype, tag="xt_ot", bufs=2, name="xt")
                for to in range(TT):
                    xstage = pool.tile([P, DIM], F32, tag="xstage", bufs=2)
                    nc.sync.dma_start(xstage[:], x[b, to * P : (to + 1) * P, :])
                    for fo in range(KT):
                        tps = ps.tile([P, QC], F32, tag="mm", bufs=2, name="tps")
                        nc.tensor.transpose(
                            tps[:, :P], xstage[:, fo * P : (fo + 1) * P], ident
                        )
                        nc.vector.tensor_copy(
                            xt[:, fo, to * P : (to + 1) * P], tps[:, :P]
                        )

                # ---- V natural + ones layout ------------------------------
                for to in range(TT):
                    for ch, cw in ((0, 512), (1, 256)):
                        vps = ps.tile([P, QC], F32, tag="mm", bufs=2, name="vps")
                        for k in range(KT):
                            nc.tensor.matmul(
                                vps[:, :cw],
                                xt[:, k, to * P : (to + 1) * P],
                                wv_sb[:, k, ch * 512 : ch * 512 + cw],
                                start=(k == 0),
                                stop=(k == KT - 1),
                            )
                        # scatter heads into pair-padded blocks (+bias)
                        npr = cw // (2 * HD)  # pairs in this chunk (4 then 2)
                        pr0 = ch * 4          # first pair in this chunk
                        for par in (0, 1):    # even/odd head of each pair
                            src = vps[:, :cw].rearrange(
                                "p (np two w) -> p np two w", two=2, w=HD
                            )[:, :, par, :]
                            bsrc = bv_b[:, ch * 512 : ch * 512 + cw].rearrange(
                                "p (np two w) -> p np two w", two=2, w=HD
                            )[:, :, par, :]
                            off = 96 if par else 0
                            dst = v_ext[:, to, :].rearrange(
                                "p (np w) -> p np w", w=PAIRW
                            )[:, pr0 : pr0 + npr, off : off + HD]
                            nc.vector.scalar_tensor_tensor(
                                out=dst,
                                in0=src,
                                scalar=1.0,
                                in1=bsrc,
                                op0=mybir.AluOpType.mult,
                                op1=mybir.AluOpType.add,
                            )

                # ---- OT buffer for this batch -----------------------------
                ot = pool.tile([P, KT, N], mm_dtype, tag="xt_ot", bufs=2, name="ot")

                # ---- per head-pair: QT/KT proj then attention -------------
                for po in range(NPAIR):
                    wqt = pool.tile([P, KT, P], mm_dtype, tag="wqt", bufs=2)
                    wkt = pool.tile([P, KT, P], mm_dtype, tag="wkt", bufs=2)
                    for k in range(KT):
                        wdma(wqt[:, k], wq_r[:, k, po * P : (po + 1) * P])
                        wdma(wkt[:, k], wk_r[:, k, po * P : (po + 1) * P])

                    qt_t = pool.tile([P, N], mm_dtype, tag="qt", bufs=2)
                    kt_t = pool.tile([P, N], mm_dtype, tag="kt", bufs=2)
                    for dst_t, w_t, bias in (
                        (qt_t, wqt, bq_sb),
                        (kt_t, wkt, bk_sb),
                    ):
                        for qs in range(N // QC):
                            pps = ps.tile([P, QC], F32, tag="mm", bufs=2, name="pps")
                            for k in range(KT):
                                nc.tensor.matmul(
                                    pps[:],
                                    w_t[:, k, :],
                                    xt[:, k, qs * QC : (qs + 1) * QC],
                                    start=(k == 0),
                                    stop=(k == KT - 1),
                                )
                            nc.vector.tensor_scalar_add(
                                dst_t[:, qs * QC : (qs + 1) * QC],
                                pps[:],
                                bias[:, po : po + 1],
                            )

                    pb = po * PAIRW
                    for qc in range(N // QC):
                        qsl = slice(qc * QC, (qc + 1) * QC)
                        oa = ps.tile([P, QC], F32, tag="oa", bufs=1, name="oa")
                        ob = ps.tile([P, QC], F32, tag="ob", bufs=1, name="ob")
                        for g in range(TT // 2):
                            st0 = ps.tile([P, 2 * QC], F32, tag="st", bufs=2, name="st0")
                            st1 = ps.tile([P, 2 * QC], F32, tag="st", bufs=2, name="st1")
                            for j in range(2):
                                kb = 2 * g + j
                                ksl = slice(kb * P, (kb + 1) * P)
                                nc.tensor.matmul(
                                    st0[:, j * QC : (j + 1) * QC],
                                    kt_t[0:64, ksl],
                                    qt_t[0:64, qsl],
                                    start=True,
                                    stop=True,
                                )
                                nc.tensor.matmul(
                                    st1[:, j * QC : (j + 1) * QC],
                                    kt_t[64:128, ksl],
                                    qt_t[64:128, qsl],
                                    start=True,
                                    stop=True,
                                )
                            pt0 = pool.tile([P, 2 * QC], mm_dtype, tag="pt0", bufs=2)
                            pt1 = pool.tile([P, 2 * QC], mm_dtype, tag="pt1", bufs=2)
                            nc.scalar.activation(
                                pt0[:], st0[:], mybir.ActivationFunctionType.Exp,
                                scale=SCALE,
                            )
                            nc.scalar.activation(
                                pt1[:], st1[:], mybir.ActivationFunctionType.Exp,
                                scale=SCALE,
                            )
                            for j in range(2):
                                kb = 2 * g + j
                                first = g == 0 and j == 0
                                last = g == TT // 2 - 1 and j == 1
                                nc.tensor.matmul(
                                    oa[0:65, :],
                                    v_ext[:, kb, pb : pb + 65],
                                    pt0[:, j * QC : (j + 1) * QC],
                                    start=first,
                                    stop=last,
                                )
                                nc.tensor.matmul(
                                    ob[:, :],
                                    v_ext[:, kb, pb + 32 : pb + 160],
                                    pt1[:, j * QC : (j + 1) * QC],
                                    start=first,
                                    stop=last,
                                )
                        # epilogue: copy psum out early (frees oa/ob banks),
                        # then normalize by the ones-row sums
                        osb_a = pool.tile([P, QC], F32, tag="osb_a", bufs=2)
                        osb_b = pool.tile([P, QC], F32, tag="osb_b", bufs=2)
                        nc.vector.tensor_copy(osb_a[0:65, :], oa[0:65, :])
                        nc.vector.tensor_copy(osb_b[64:128, :], ob[64:128, :])
                        rrow = pool.tile([P, QC], F32, tag="rrow", bufs=2)
                        nc.vector.reciprocal(rrow[64:65, :], osb_a[64:65, :])
                        nc.vector.reciprocal(rrow[32:33, :], ob[32:33, :])
                        rb = pool.tile([P, QC], F32, tag="rb", bufs=2)
                        nc.gpsimd.partition_broadcast(rb[0:64, :], rrow[64:65, :])
                        nc.gpsimd.partition_broadcast(rb[64:128, :], rrow[32:33, :])
                        nc.vector.tensor_mul(
                            ot[0:64, po, qsl], osb_a[0:64, :], rb[0:64, :]
                        )
                        nc.vector.tensor_mul(
                            ot[64:128, po, qsl], osb_b[64:128, :], rb[64:128, :]
                        )

                # ---- Y = OT^T @ Wo + bo  (natural layout) ------------------
                for to in range(TT):
                    ystage = pool.tile([P, DIM], F32, tag="ystage", bufs=2)
                    for ch, cw in ((0, 512), (1, 256)):
                        yps = ps.tile([P, QC], F32, tag="mm", bufs=2, name="yps")
                        for k in range(KT):
                            nc.tensor.matmul(
                                yps[:, :cw],
                                ot[:, k, to * P : (to + 1) * P],
                                wo_sb[:, k, ch * 512 : ch * 512 + cw],
                                start=(k == 0),
                                stop=(k == KT - 1),
                            )
                        nc.vector.scalar_tensor_tensor(
                            out=ystage[:, ch * 512 : ch * 512 + cw],
                            in0=yps[:, :cw],
                            scalar=1.0,
                            in1=bo_b[:, ch * 512 : ch * 512 + cw],
                            op0=mybir.AluOpType.mult,
                            op1=mybir.AluOpType.add,
                        )
                    nc.sync.dma_start(
                        out[b, to * P : (to + 1) * P, :], ystage[:]
                    )

    nc.finalize()
    return nc


def _run(inputs: dict, mm_dtype=None, trace: bool = False):
    if mm_dtype is None:
        mm_dtype = mybir.dt.float32r
    key = str(mm_dtype)
    if key not in _cache:
        _cache[key] = build(mm_dtype)
    nc = _cache[key]

    x = np.ascontiguousarray(inputs["inputs"], dtype=np.float32)
    shared = {
        k: np.ascontiguousarray(inputs[k], dtype=np.float32)
        for k in ("Wq", "bq", "Wk", "bk", "Wv", "bv", "Wo", "bo")
    }
    in_maps = [
        {"inputs": x[c * BL : (c + 1) * BL], **shared} for c in range(NCORES)
    ]
    res = run_bass_kernel_spmd(nc, in_maps, list(range(NCORES)), trace=trace)
    full = np.concatenate([res.results[c]["out"] for c in range(NCORES)], axis=0)
    return full, res


def kernel(**inputs) -> np.ndarray:
    out, _ = _run(inputs)
    return out
